# revision 40
# baseline (speedup 1.0000x reference)
"""Trainium2 Bass kernel for nn_DetectionLoss (B=16, N=25000, M=64).

Strategy (v4 — fp16 match pipeline, software-pipelined, short tail):
- Data-parallel: 8 cores x 2 images each; host shards batch and averages.
- Greedy match reformulated as per-GT argmax (exact). Ranking uses
  q = inter/(area_p+area_t), monotone in iou; thr is q > 1/6.
- Match DECISIONS tolerate fp16 (numpy sim: rel err 3.5e-4; gate 2e-2).
  The loss tail stays exact f32 via DRAM gathers of the raw preds.
- Bulk pairwise in fp16 for the DVE 2x_1p mode; broadcast target operands
  are materialized once per image as [P, M, UG] replicated tiles
  (log-doubling TensorCopy at 4x) so the mode is not lost to stride-0 APs.
- Engines: DVE minmax/inter/q/macc (f16 2x) + recip (f32); Pool ssum +
  dx/dy; Act relu + rsc f32->f16. Emission is wave-skewed (2-wave software
  pipeline); image 0's tail is interleaved into image 1's bulk waves.
- Tail avoids MaxIndex: first-occurrence argmax via is_equal + iota + min
  reduce (exact same tie-break as jnp.argmax on identical fp16 values).
  Focal correction f1-f0 is precomputed for all preds during bulk and
  gathered per matched GT. Cross-partition sums go through PE matmuls.
"""

import numpy as np

B, N, M = 16, 25000, 64
P = 128
SLOTS = 196
IMGS_PER_CORE = 2
N_CORES = 8
UG = 28
NGROUPS = SLOTS // UG  # 7

PAD_PART = 127
PAD_START = N - PAD_PART * SLOTS   # 108

_cache = {}


def _build(debug_dumps=False):
    import concourse.bass as bass
    import concourse.bacc as bacc
    import concourse.mybir as mybir
    from concourse import tile
    from concourse.bass import IndirectOffsetOnAxis
    from concourse.masks import make_identity

    f32 = mybir.dt.float32
    f16 = mybir.dt.float16
    u32 = mybir.dt.uint32
    i32 = mybir.dt.int32
    Alu = mybir.AluOpType
    Act = mybir.ActivationFunctionType
    X = mybir.AxisListType.X

    nc = bacc.Bacc("TRN2", target_bir_lowering=False, debug=False,
                   num_devices=N_CORES)

    preds_d = nc.dram_tensor("preds", [IMGS_PER_CORE, N, 5], f32, kind="ExternalInput")
    targets_d = nc.dram_tensor("targets", [IMGS_PER_CORE, M, 4], f32, kind="ExternalInput")
    out_d = nc.dram_tensor("out", [IMGS_PER_CORE], f32, kind="ExternalOutput")
    q_d = [nc.dram_tensor(f"q_scratch{b}", [P * M, SLOTS], f16)
           for b in range(IMGS_PER_CORE)]
    D_d = [nc.dram_tensor(f"d_scratch{b}", [P * SLOTS, 1], f32)
           for b in range(IMGS_PER_CORE)]

    EPS = np.float32(1e-7)
    C_4PI2 = np.float32(4.0 / (np.pi ** 2))
    BIG = float(2 ** 18)
    SP_SEED = [0.041064513, -0.156028432, 0.304672365, -0.496368282, 0.999887926]
    # A&S 4.4.49: atan(r)/r, poly in r^2, |err|<=1e-5 on [0,1]; plenty for
    # the aspect-ratio term (weight ~4/pi^2 * small delta)
    AT_POLY = [0.0208351, -0.0851330, 0.1801410, -0.3302995, 0.9998660]

    with tile.TileContext(nc) as tc:
        with (
            tc.tile_pool(name="qpool", bufs=2) as qpool,
            tc.tile_pool(name="ppool", bufs=2) as ppool,
            tc.tile_pool(name="der", bufs=2) as der,
            tc.tile_pool(name="rep", bufs=2) as rep,
            tc.tile_pool(name="grp", bufs=3) as grp,
            tc.tile_pool(name="qch", bufs=3) as qch,
            tc.tile_pool(name="ssp", bufs=2) as ssp,
            tc.tile_pool(name="inp", bufs=2) as inp,
            tc.tile_pool(name="mac", bufs=2) as mac,
            tc.tile_pool(name="sml", bufs=2) as sml,
            tc.tile_pool(name="cst", bufs=1) as cst,
            tc.tile_pool(name="psum", bufs=1,
                         space=bass.MemorySpace.PSUM) as psum,
        ):
            # constants
            iota_p64 = cst.tile([M, 1], i32, tag="iota_p64")
            nc.gpsimd.iota(iota_p64[:], pattern=[[1, 1]], base=0, channel_multiplier=1)
            iscr = cst.tile([M, SLOTS], i32, tag="iscr")
            nc.gpsimd.iota(iscr[:, :M], pattern=[[1, M]], base=0,
                           channel_multiplier=0)
            iota_p64f = cst.tile([M, 1], f32, tag="iota_p64f")
            nc.vector.tensor_copy(iota_p64f[:], iota_p64[:])
            iota_f64f = cst.tile([M, M], f32, tag="iota_f64f")
            nc.vector.tensor_copy(iota_f64f[:], iscr[:, :M])
            ltmask = cst.tile([M, M], f32, tag="ltmask")
            nc.vector.tensor_scalar(ltmask[:], iota_f64f[:], iota_p64f[:], None,
                                    op0=Alu.is_lt)
            iotaPf = cst.tile([M, P], f32, tag="iotaPf")
            nc.gpsimd.iota(iscr[:, :P], pattern=[[1, P]], base=0,
                           channel_multiplier=0)
            nc.vector.tensor_copy(iotaPf[:], iscr[:, :P])
            iotaSf = cst.tile([M, SLOTS], f32, tag="iotaSf")
            nc.gpsimd.iota(iscr[:], pattern=[[1, SLOTS]], base=0,
                           channel_multiplier=0)
            nc.vector.tensor_copy(iotaSf[:], iscr[:])
            ones_row = cst.tile([1, P], f32, tag="ones_row")
            nc.gpsimd.memset(ones_row[:], 1.0)
            ones_col64 = cst.tile([M, 1], f32, tag="ones_col64")
            nc.gpsimd.memset(ones_col64[:], 1.0)
            ones_col128 = cst.tile([P, 1], f32, tag="ones_col128")
            nc.gpsimd.memset(ones_col128[:], 1.0)
            ident = cst.tile([P, P], f32, tag="ident")
            make_identity(nc, ident[:])

            st = [dict() for _ in range(IMGS_PER_CORE)]

            def pesum(x_ap, n, tag):
                """sum over partitions of [n,1] x -> [1,1] f32 tile"""
                ps = psum.tile([1, 1], f32, tag="ps_sum", name="ps_sum")
                ones = ones_col64 if n == M else ones_col128
                nc.tensor.matmul(ps[:], x_ap, ones[:], start=True, stop=True)
                out = sml.tile([1, 1], f32, tag="sum_" + tag, name="sum_" + tag)
                nc.vector.tensor_copy(out[:], ps[:])
                return out

            # ---------------- loads for both images up front --------------
            for b in range(IMGS_PER_CORE):
                s = st[b]
                predsI = ppool.tile([P, SLOTS, 5], f32, tag="predsI", name="predsI")
                nc.gpsimd.memset(predsI[:, PAD_START:, 0:2], 50.0)
                nc.gpsimd.memset(predsI[:, PAD_START:, 2:4], 1e-4)
                nc.gpsimd.memset(predsI[:, PAD_START:, 4:5], -80.0)
                src = preds_d.ap()[b].rearrange("n c -> (n c)")
                nc.sync.dma_start(
                    predsI[:PAD_PART],
                    src[: PAD_PART * SLOTS * 5].rearrange("(p f) -> p f", p=PAD_PART)
                    .rearrange("p (s c) -> p s c", c=5))
                nc.sync.dma_start(
                    predsI[PAD_PART:, :PAD_START],
                    src[PAD_PART * SLOTS * 5:].rearrange("(p s c) -> p s c", p=1, c=5))
                s["predsI"] = predsI
                tg = sml.tile([M, 4], f32, tag="tg", name="tg")
                nc.sync.dma_start(tg[:], targets_d.ap()[b])
                trow = sml.tile([1, M, 4], f32, tag="trow", name="trow")
                nc.sync.dma_start(trow[:], targets_d.ap()[b].unsqueeze(0))
                s["tg"] = tg
                s["trow"] = trow

            # ---------------- prelude -------------------------------------
            def prelude(b):
                s = st[b]
                predsI = s["predsI"]
                wc = der.tile([P, SLOTS], f32, tag="wc", name="wc")
                hc = der.tile([P, SLOTS], f32, tag="hc", name="hc")
                x1p = der.tile([P, SLOTS], f16, tag="x1p", name="x1p")
                x2p = der.tile([P, SLOTS], f16, tag="x2p", name="x2p")
                y1p = der.tile([P, SLOTS], f16, tag="y1p", name="y1p")
                y2p = der.tile([P, SLOTS], f16, tag="y2p", name="y2p")
                apred = der.tile([P, SLOTS], f32, tag="apred", name="apred")
                nc.vector.tensor_scalar_max(wc[:], predsI[:, :, 2], 1e-4)
                nc.vector.tensor_scalar_max(hc[:], predsI[:, :, 3], 1e-4)
                nc.vector.tensor_tensor(apred[:], wc[:], hc[:], op=Alu.mult)
                nc.vector.tensor_scalar_mul(wc[:], wc[:], 0.5)
                nc.vector.tensor_tensor(x1p[:], predsI[:, :, 0], wc[:],
                                        op=Alu.subtract)
                nc.vector.tensor_tensor(x2p[:], predsI[:, :, 0], wc[:],
                                        op=Alu.add)
                nc.vector.tensor_scalar_mul(hc[:], hc[:], 0.5)
                nc.vector.tensor_tensor(y1p[:], predsI[:, :, 1], hc[:],
                                        op=Alu.subtract)
                nc.vector.tensor_tensor(y2p[:], predsI[:, :, 1], hc[:],
                                        op=Alu.add)
                s.update(x1p=x1p, x2p=x2p, y1p=y1p, y2p=y2p, apred=apred)

                trow = s["trow"]
                atrow = sml.tile([1, M, 2], f32, tag="atrow", name="atrow")
                nc.vector.tensor_sub(atrow[:, :, 0], trow[:, :, 2], trow[:, :, 0])
                nc.vector.tensor_sub(atrow[:, :, 1], trow[:, :, 3], trow[:, :, 1])
                nc.vector.tensor_tensor(atrow[:, :, 0], atrow[:, :, 0],
                                        atrow[:, :, 1], op=Alu.mult)
                coord16 = []
                for ci in range(4):
                    pt = psum.tile([P, M], f32, tag="bcast_ps", name="bcast_ps")
                    nc.tensor.matmul(pt[:], ones_row[:], trow[:, :, ci],
                                     start=True, stop=True)
                    c16 = rep.tile([P, M], f16, tag=f"tb16_{ci}", name=f"tb16_{ci}")
                    nc.scalar.copy(c16[:], pt[:])
                    coord16.append(c16)
                pt = psum.tile([P, M], f32, tag="bcast_ps", name="bcast_ps")
                nc.tensor.matmul(pt[:], ones_row[:], atrow[:, :, 0],
                                 start=True, stop=True)
                atB = rep.tile([P, M], f32, tag="atB", name="atB")
                nc.scalar.copy(atB[:], pt[:])
                s["atB"] = atB
                reps = []
                for ci in range(4):
                    cp = (nc.vector.tensor_copy if (b == 0 or ci >= 2)
                          else nc.scalar.copy)
                    r = rep.tile([P, M, UG], f16, tag=f"rep_{ci}", name=f"rep_{ci}")
                    cp(r[:, :, 0:1], coord16[ci][:].unsqueeze(2))
                    k = 1
                    while k < UG:
                        step = min(k, UG - k)
                        cp(r[:, :, k:k + step], r[:, :, 0:step])
                        k += step
                    reps.append(r)
                s["reps"] = reps
                s["q"] = qpool.tile([P, M, SLOTS], f16, tag="q", name="q")
                s["macc"] = mac.tile([P, M, UG], f16, tag="macc", name="macc")
                s["grp"] = {}
                s["qch"] = {}
                s["ssum"] = {}

            # ---------------- bulk wave stages ----------------------------
            def stage_mm(b, g):
                s = st[b]
                sl = slice(g * UG, (g + 1) * UG)
                x1tR, y1tR, x2tR, y2tR = s["reps"]

                def pv16(t):
                    return t[:, sl].unsqueeze(1).to_broadcast([P, M, UG])

                ltx = grp.tile([P, M, UG], f16, tag="ltx", name="ltx")
                rbx = grp.tile([P, M, UG], f16, tag="rbx", name="rbx")
                lty = grp.tile([P, M, UG], f16, tag="lty", name="lty")
                rby = grp.tile([P, M, UG], f16, tag="rby", name="rby")
                rsc16 = qch.tile([P, M, UG], f16, tag="rsc16", name="rsc16")
                intr = inp.tile([P, M, UG], f16, tag="intr", name="intr")
                s["grp"][g] = [ltx, rbx, lty, rby, None, rsc16, intr]
                nc.vector.tensor_tensor(ltx[:], pv16(s["x1p"]), x1tR[:], op=Alu.max)
                nc.vector.tensor_tensor(rbx[:], pv16(s["x2p"]), x2tR[:], op=Alu.min)
                nc.vector.tensor_tensor(lty[:], pv16(s["y1p"]), y1tR[:], op=Alu.max)
                nc.vector.tensor_tensor(rby[:], pv16(s["y2p"]), y2tR[:], op=Alu.min)

            def stage_ssum(b, g):
                s = st[b]
                sl = slice(g * UG, (g + 1) * UG)
                ssum = ssp.tile([P, M, UG], f32, tag="ssum", name="ssum")
                s["grp"][g][4] = ssum
                eng = nc.vector if (g < 3 if b == 0 else g < 1) else nc.gpsimd
                eng.tensor_tensor(
                    ssum[:],
                    s["apred"][:, sl].unsqueeze(1).to_broadcast([P, M, UG]),
                    s["atB"][:].unsqueeze(2).to_broadcast([P, M, UG]),
                    op=Alu.add)

            def stage_mid(b, g):
                s = st[b]
                ltx, rbx, lty, rby, ssum, rsc16, intr = s["grp"][g]
                nc.gpsimd.tensor_tensor(rbx[:], rbx[:], ltx[:], op=Alu.subtract)
                if b == 0 and g == NGROUPS - 1:
                    nc.vector.tensor_tensor(rby[:], rby[:], lty[:], op=Alu.subtract)
                else:
                    nc.gpsimd.tensor_tensor(rby[:], rby[:], lty[:], op=Alu.subtract)
                nc.scalar.activation(ltx[:], rbx[:], Act.Relu)

            def stage_recip(b, g):
                s = st[b]
                ltx, rbx, lty, rby, ssum, rsc16, intr = s["grp"][g]
                nc.vector.reciprocal_approx_fast(ssum[:], ssum[:])
                nc.scalar.copy(rsc16[:], ssum[:])

            def stage_fin(b, g):
                s = st[b]
                ltx, rbx, lty, rby, ssum, rsc16, intr = s["grp"][g]
                nc.vector.tensor_tensor(intr[:], ltx[:], rby[:], op=Alu.mult)
                s["qch"][g] = (rsc16, intr)
                del s["grp"][g]

            def stage_finq(b, g):
                s = st[b]
                rsc16, intr = s["qch"][g]
                sl = slice(g * UG, (g + 1) * UG)
                qs = s["q"][:, :, sl]
                nc.vector.tensor_tensor(qs, intr[:], rsc16[:], op=Alu.mult)
                if g == 0:
                    nc.vector.tensor_copy(s["macc"][:], qs)
                else:
                    nc.vector.tensor_tensor(s["macc"][:], s["macc"][:], qs,
                                            op=Alu.max)
                del s["qch"][g]

            # ---------------- focal bulk (chunked) ------------------------
            def focal_bulk_chunks(b):
                s = st[b]
                predsI = s["predsI"]
                tl = lambda t: der.tile([P, SLOTS], f32, tag="fb" + t, name="fb" + t)
                sg_, sp_, u_, w_, z_, e_ = (tl("sg"), tl("sp"), tl("u"),
                                            tl("w"), tl("z"), tl("e"))
                conf = predsI[:, :, 4]

                def c0():
                    nc.scalar.activation(e_[:], conf, Act.Exp, scale=-1.0)
                    nc.vector.tensor_scalar_add(e_[:], e_[:], 1.0)
                    nc.vector.reciprocal(sg_[:], e_[:])
                    nc.vector.tensor_scalar_mul(u_[:], conf, -1.0)
                    nc.vector.tensor_tensor(u_[:], u_[:], conf, op=Alu.max)
                    nc.scalar.activation(u_[:], u_[:], Act.Exp, scale=-1.0)
                    nc.vector.tensor_scalar_add(w_[:], u_[:], 1.0)

                def c1():
                    nc.vector.tensor_scalar(z_[:], u_[:], float(SP_SEED[0]),
                                            float(SP_SEED[1]), op0=Alu.mult,
                                            op1=Alu.add)
                    for coef in SP_SEED[2:]:
                        nc.gpsimd.tensor_tensor(z_[:], z_[:], u_[:], op=Alu.mult)
                        nc.vector.tensor_scalar_add(z_[:], z_[:], float(coef))
                    nc.gpsimd.tensor_tensor(z_[:], z_[:], u_[:], op=Alu.mult)

                def newton():
                    nc.scalar.activation(e_[:], z_[:], Act.Exp, scale=-1.0)
                    nc.gpsimd.tensor_tensor(e_[:], w_[:], e_[:], op=Alu.mult)
                    nc.gpsimd.tensor_tensor(z_[:], z_[:], e_[:], op=Alu.add)
                    nc.vector.tensor_scalar_add(z_[:], z_[:], -1.0)

                def c3():
                    nc.scalar.activation(sp_[:], conf, Act.Relu)
                    nc.vector.tensor_add(sp_[:], sp_[:], z_[:])
                    f0 = z_  # f0 = sg^2 * sp
                    nc.gpsimd.tensor_tensor(f0[:], sg_[:], sg_[:], op=Alu.mult)
                    nc.gpsimd.tensor_tensor(f0[:], f0[:], sp_[:], op=Alu.mult)
                    frow = sml.tile([P, 1], f32, tag="frow", name="frow")
                    nc.vector.tensor_reduce(frow[:], f0[:], axis=X, op=Alu.add)
                    s["fsum"] = pesum(frow[:], P, f"fs{b}")

                def c4():
                    # D = 0.25*(1-sg)^2*(sp-conf) - 0.75*f0  (focal1 - focal0)
                    t1_, t2_ = u_, e_
                    nc.vector.tensor_scalar(t1_[:], sg_[:], -1.0, 1.0,
                                            op0=Alu.mult, op1=Alu.add)
                    nc.vector.tensor_tensor(t1_[:], t1_[:], t1_[:], op=Alu.mult)
                    nc.gpsimd.tensor_tensor(t2_[:], sp_[:], conf, op=Alu.subtract)
                    nc.gpsimd.tensor_tensor(t1_[:], t1_[:], t2_[:], op=Alu.mult)
                    nc.vector.tensor_scalar_mul(t1_[:], t1_[:], 0.25)
                    nc.vector.tensor_scalar_mul(z_[:], z_[:], 0.75)  # z_ holds f0
                    nc.vector.tensor_tensor(t1_[:], t1_[:], z_[:], op=Alu.subtract)
                    nc.sync.dma_start(
                        D_d[b].ap().rearrange("(p s) o -> p s o", p=P),
                        t1_[:].unsqueeze(2))

                return [c0, c1, newton, newton, c3, c4]

            # ---------------- tail pieces ---------------------------------
            def tail_pieces(b):
                s = st[b]
                tg = s["tg"]
                h = {}
                t1 = lambda tag: sml.tile([M, 1], f32, tag=tag, name=tag)
                t2_ = lambda tag: sml.tile([M, 2], f32, tag=tag, name=tag)

                def p0():
                    if "shipped" not in st[b]:
                        nc.sync.dma_start(
                            q_d[b].ap().rearrange("(p m) s -> p m s", p=P),
                            st[b]["q"][:])
                        st[b]["shipped"] = True
                    m1 = sml.tile([P, M], f32, tag="m1", name="m1")
                    nc.vector.tensor_reduce(m1[:], s["macc"][:], axis=X, op=Alu.max)
                    m1tp = psum.tile([M, P], f32, tag="m1tp", name="m1tp")
                    nc.tensor.transpose(m1tp[:], m1[:], ident[:])
                    m1t = sml.tile([M, P], f32, tag="m1t", name="m1t")
                    nc.vector.tensor_copy(m1t[:], m1tp[:])
                    gmax = t1("gmax")
                    nc.vector.tensor_reduce(gmax[:], m1t[:], axis=X, op=Alu.max)
                    # p* = first partition hitting gmax: is_eq*(-BIG)+iota, min
                    nc.vector.tensor_scalar(m1t[:], m1t[:], gmax[:], -BIG,
                                            op0=Alu.is_equal, op1=Alu.mult)
                    nc.vector.tensor_tensor(m1t[:], m1t[:], iotaPf[:], op=Alu.add)
                    pstar = t1("pstar")
                    nc.vector.tensor_reduce(pstar[:], m1t[:], axis=X, op=Alu.min)
                    nc.vector.tensor_scalar_add(pstar[:], pstar[:], BIG)
                    pu = sml.tile([M, 1], u32, tag="pu", name="pu")
                    nc.vector.tensor_copy(pu[:], pstar[:])
                    rowoff = sml.tile([M, 1], u32, tag="rowoff", name="rowoff")
                    nc.vector.tensor_scalar_mul(rowoff[:], pu[:], M)
                    nc.vector.tensor_tensor(rowoff[:], rowoff[:],
                                            iota_p64[:].bitcast(u32), op=Alu.add)
                    qrow16 = sml.tile([M, SLOTS], f16, tag="qrow16", name="qrow16")
                    nc.gpsimd.indirect_dma_start(
                        out=qrow16[:], out_offset=None,
                        in_=q_d[b].ap(),
                        in_offset=IndirectOffsetOnAxis(ap=rowoff[:], axis=0))
                    h.update(gmax=gmax, pstar=pstar, qrow16=qrow16)

                def p1():
                    qrow = sml.tile([M, SLOTS], f32, tag="qrow", name="qrow")
                    nc.vector.tensor_copy(qrow[:], h["qrow16"][:])
                    # c* = first slot hitting gmax within the gathered row
                    nc.vector.tensor_scalar(qrow[:], qrow[:], h["gmax"][:], -BIG,
                                            op0=Alu.is_equal, op1=Alu.mult)
                    nc.vector.tensor_tensor(qrow[:], qrow[:], iotaSf[:], op=Alu.add)
                    cstar = t1("cstar")
                    nc.vector.tensor_reduce(cstar[:], qrow[:], axis=X, op=Alu.min)
                    nc.vector.tensor_scalar_add(cstar[:], cstar[:], BIG)
                    nstar_f = t1("nstar_f")
                    nc.vector.tensor_scalar(nstar_f[:], h["pstar"][:], float(SLOTS),
                                            cstar[:], op0=Alu.mult, op1=Alu.add)
                    nstar = sml.tile([M, 1], u32, tag="nstar", name="nstar")
                    nc.vector.tensor_copy(nstar[:], nstar_f[:])
                    thr = t1("thr")
                    nc.vector.tensor_scalar(thr[:], h["gmax"][:], float(1.0 / 6.0),
                                            None, op0=Alu.is_gt)
                    # start g5 + D gathers (independent of dedup)
                    g5 = sml.tile([M, 5], f32, tag="g5", name="g5")
                    nrow = sml.tile([M, 1], u32, tag="nrow", name="nrow")
                    nc.vector.tensor_scalar_add(nrow[:], nstar[:], b * N)
                    nc.gpsimd.indirect_dma_start(
                        out=g5[:], out_offset=None,
                        in_=preds_d.ap().rearrange("b n c -> (b n) c"),
                        in_offset=IndirectOffsetOnAxis(ap=nrow[:], axis=0))
                    Dg = sml.tile([M, 1], f32, tag="Dg", name="Dg")
                    nc.gpsimd.indirect_dma_start(
                        out=Dg[:], out_offset=None,
                        in_=D_d[b].ap(),
                        in_offset=IndirectOffsetOnAxis(ap=nstar[:], axis=0))
                    h.update(nstar_f=nstar_f, thr=thr, g5=g5, Dg=Dg)

                def p2():
                    nstar_f, thr = h["nstar_f"], h["thr"]
                    pair = sml.tile([M, 2], f32, tag="pair", name="pair")
                    nc.vector.tensor_copy(pair[:, 0:1], nstar_f[:])
                    nc.vector.tensor_copy(pair[:, 1:2], thr[:])
                    pairT_ps = psum.tile([1, 2, M], f32, tag="pairT_ps",
                                         name="pairT_ps")
                    nc.tensor.transpose(pairT_ps[:, 0], pair[:, 0:1], ident[:M, :M])
                    nc.tensor.transpose(pairT_ps[:, 1], pair[:, 1:2], ident[:M, :M])
                    pairT = sml.tile([1, 2, M], f32, tag="pairT", name="pairT")
                    nc.vector.tensor_copy(pairT[:], pairT_ps[:])
                    rowB = sml.tile([M, M, 2], f32, tag="rowB", name="rowB")
                    ptb = psum.tile([M, M, 2], f32, tag="ptb", name="ptb")
                    nc.tensor.matmul(ptb[:, :, 0], ones_row[:, :M], pairT[:, 0],
                                     start=True, stop=True)
                    nc.tensor.matmul(ptb[:, :, 1], ones_row[:, :M], pairT[:, 1],
                                     start=True, stop=True)
                    nc.scalar.copy(rowB[:], ptb[:])
                    eq = sml.tile([M, M], f32, tag="eq", name="eq")
                    nc.vector.tensor_scalar(eq[:], rowB[:, :, 0], nstar_f[:], None,
                                            op0=Alu.is_equal)
                    nc.vector.tensor_tensor(eq[:], eq[:], rowB[:, :, 1], op=Alu.mult)
                    nc.vector.tensor_tensor(eq[:], eq[:], ltmask[:], op=Alu.mult)
                    blocked = t1("blocked")
                    nc.vector.tensor_reduce(blocked[:], eq[:], axis=X, op=Alu.max)
                    ok = t1("ok")
                    nc.vector.tensor_scalar(ok[:], blocked[:], -1.0, 1.0,
                                            op0=Alu.mult, op1=Alu.add)
                    nc.vector.tensor_tensor(ok[:], ok[:], thr[:], op=Alu.mult)
                    h["ok"] = ok

                def p3():
                    # lane-packed ciou: lane 0 = x, lane 1 = y
                    g5 = h["g5"]
                    tgA, tgB = tg[:, 0:2], tg[:, 2:4]     # (tx1,ty1), (tx2,ty2)
                    wh2 = t2_("wh2")
                    nc.vector.tensor_scalar_max(wh2[:], g5[:, 2:4], 1e-4)
                    nc.vector.tensor_scalar_mul(wh2[:], wh2[:], 0.5)
                    c1_ = t2_("c1_")
                    c2_ = t2_("c2_")
                    nc.vector.tensor_tensor(c1_[:], g5[:, 0:2], wh2[:],
                                            op=Alu.subtract)
                    nc.vector.tensor_tensor(c2_[:], g5[:, 0:2], wh2[:], op=Alu.add)
                    lt2 = t2_("lt2")
                    rb2 = t2_("rb2")
                    nc.vector.tensor_tensor(lt2[:], c1_[:], tgA, op=Alu.max)
                    nc.vector.tensor_tensor(rb2[:], c2_[:], tgB, op=Alu.min)
                    nc.vector.tensor_tensor(rb2[:], rb2[:], lt2[:], op=Alu.subtract)
                    nc.vector.tensor_scalar_max(rb2[:], rb2[:], 0.0)
                    ginter = t1("ginter")
                    nc.vector.tensor_tensor(ginter[:], rb2[:, 0:1], rb2[:, 1:2],
                                            op=Alu.mult)
                    whp = t2_("whp")
                    wht = t2_("wht")
                    nc.vector.tensor_tensor(whp[:], c2_[:], c1_[:], op=Alu.subtract)
                    nc.vector.tensor_tensor(wht[:], tgB, tgA, op=Alu.subtract)
                    gu = t1("gu")
                    ga = t1("ga")
                    nc.vector.tensor_tensor(gu[:], whp[:, 0:1], whp[:, 1:2],
                                            op=Alu.mult)
                    nc.vector.tensor_tensor(ga[:], wht[:, 0:1], wht[:, 1:2],
                                            op=Alu.mult)
                    nc.vector.tensor_add(gu[:], gu[:], ga[:])
                    nc.vector.tensor_sub(gu[:], gu[:], ginter[:])
                    giou = t1("giou")
                    nc.vector.tensor_scalar_add(gu[:], gu[:], float(EPS))
                    nc.vector.reciprocal(gu[:], gu[:])
                    nc.vector.tensor_tensor(giou[:], ginter[:], gu[:], op=Alu.mult)
                    h.update(c1_=c1_, c2_=c2_, giou=giou, whp=whp, wht=wht)

                def p4():
                    c1_, c2_ = h["c1_"], h["c2_"]
                    giou, whp, wht = h["giou"], h["whp"], h["wht"]
                    tgA, tgB = tg[:, 0:2], tg[:, 2:4]
                    e1 = t2_("e1")
                    e2 = t2_("e2")
                    nc.vector.tensor_tensor(e1[:], c1_[:], tgA, op=Alu.min)
                    nc.vector.tensor_tensor(e2[:], c2_[:], tgB, op=Alu.max)
                    nc.vector.tensor_tensor(e2[:], e2[:], e1[:], op=Alu.subtract)
                    nc.vector.tensor_tensor(e2[:], e2[:], e2[:], op=Alu.mult)
                    diag = t1("diag")
                    nc.vector.tensor_add(diag[:], e2[:, 0:1], e2[:, 1:2])
                    nc.vector.tensor_scalar_add(diag[:], diag[:], float(EPS))
                    ce2 = t2_("ce2")
                    nc.vector.tensor_tensor(ce2[:], c1_[:], c2_[:], op=Alu.add)
                    nc.vector.tensor_tensor(ce2[:], ce2[:], tgA, op=Alu.subtract)
                    nc.vector.tensor_tensor(ce2[:], ce2[:], tgB, op=Alu.subtract)
                    nc.vector.tensor_tensor(ce2[:], ce2[:], ce2[:], op=Alu.mult)
                    cent = t1("cent")
                    nc.vector.tensor_add(cent[:], ce2[:, 0:1], ce2[:, 1:2])
                    nc.vector.tensor_scalar_mul(cent[:], cent[:], 0.25)
                    diou = t1("diou")
                    nc.vector.reciprocal(diag[:], diag[:])
                    nc.vector.tensor_tensor(diou[:], cent[:], diag[:], op=Alu.mult)
                    nc.vector.tensor_sub(diou[:], diou[:], giou[:])
                    nc.vector.tensor_scalar_add(diou[:], diou[:], 1.0)
                    # v term: atan ratios packed [t, p]
                    rat = sml.tile([M, 2], f32, tag="rat", name="rat")
                    big2 = sml.tile([M, 2], i32, tag="big2", name="big2")
                    inv2 = sml.tile([M, 2], f32, tag="inv2", name="inv2")
                    s2 = sml.tile([M, 2], f32, tag="s2", name="s2")
                    ac2 = sml.tile([M, 2], f32, tag="ac2", name="ac2")
                    nc.vector.reciprocal(rat[:, 0:1], wht[:, 1:2])
                    nc.vector.tensor_tensor(rat[:, 0:1], wht[:, 0:1], rat[:, 0:1],
                                            op=Alu.mult)
                    nc.vector.reciprocal(rat[:, 1:2], whp[:, 1:2])
                    nc.vector.tensor_tensor(rat[:, 1:2], whp[:, 0:1], rat[:, 1:2],
                                            op=Alu.mult)
                    nc.vector.tensor_scalar(big2[:], rat[:], 1.0, None, op0=Alu.is_gt)
                    nc.vector.reciprocal(inv2[:], rat[:])
                    nc.vector.copy_predicated(rat[:], big2[:], inv2[:])
                    nc.vector.tensor_tensor(s2[:], rat[:], rat[:], op=Alu.mult)
                    nc.vector.tensor_scalar(ac2[:], s2[:], float(AT_POLY[0]),
                                            float(AT_POLY[1]), op0=Alu.mult,
                                            op1=Alu.add)
                    for coef in AT_POLY[2:]:
                        nc.vector.tensor_tensor(ac2[:], ac2[:], s2[:], op=Alu.mult)
                        nc.vector.tensor_scalar_add(ac2[:], ac2[:], float(coef))
                    nc.vector.tensor_tensor(ac2[:], ac2[:], rat[:], op=Alu.mult)
                    nc.vector.tensor_scalar(inv2[:], ac2[:], -1.0, float(np.pi / 2),
                                            op0=Alu.mult, op1=Alu.add)
                    nc.vector.copy_predicated(ac2[:], big2[:], inv2[:])
                    vv = t1("vv")
                    nc.vector.tensor_sub(vv[:], ac2[:, 0:1], ac2[:, 1:2])
                    nc.vector.tensor_tensor(vv[:], vv[:], vv[:], op=Alu.mult)
                    nc.vector.tensor_scalar_mul(vv[:], vv[:], float(C_4PI2))
                    av = t1("av")
                    nc.vector.tensor_scalar(av[:], giou[:], -1.0, float(1.0 + EPS),
                                            op0=Alu.mult, op1=Alu.add)
                    nc.vector.tensor_add(av[:], av[:], vv[:])
                    nc.vector.reciprocal(av[:], av[:])
                    nc.vector.tensor_tensor(av[:], av[:], vv[:], op=Alu.mult)
                    ciou = t1("ciou")
                    nc.vector.tensor_tensor(ciou[:], av[:], vv[:], op=Alu.mult)
                    nc.vector.tensor_add(ciou[:], ciou[:], diou[:])
                    ok = h["ok"]
                    nc.vector.tensor_tensor(ciou[:], ciou[:], ok[:], op=Alu.mult)
                    bsum = pesum(ciou[:], M, f"bs{b}")
                    nmatch = pesum(ok[:], M, f"nm{b}")
                    nc.vector.tensor_scalar_max(nmatch[:], nmatch[:], 1.0)
                    nc.vector.reciprocal(nmatch[:], nmatch[:])
                    box_loss = sml.tile([1, 1], f32, tag="box_loss", name="box_loss")
                    nc.vector.tensor_tensor(box_loss[:], bsum[:], nmatch[:],
                                            op=Alu.mult)
                    h["box_loss"] = box_loss

                def p5():
                    s_ = st[b]
                    mf1 = t1("mf1")
                    nc.vector.tensor_tensor(mf1[:], h["Dg"][:], h["ok"][:],
                                            op=Alu.mult)
                    dsum = pesum(mf1[:], M, f"ds{b}")
                    acc = sml.tile([1, 1], f32, tag="acc", name="acc")
                    nc.vector.tensor_scalar_mul(acc[:], s_["fsum"][:], 0.75)
                    nc.vector.tensor_add(acc[:], acc[:], dsum[:])
                    nc.vector.tensor_scalar_mul(acc[:], acc[:], float(1.0 / N))
                    nc.vector.tensor_add(acc[:], acc[:], h["box_loss"][:])
                    nc.sync.dma_start(out_d.ap()[b:b + 1],
                                      acc[:].rearrange("o m -> (o m)"))

                return [p0, p1, p2, p3, p4, p5]

            # ---------------- emission schedule ---------------------------
            # unified wave schedule: image 0 groups at waves 0..6, image 1
            # at waves 7..13 (Pool stream seamless across the transition)
            prelude(0)
            fb0 = focal_bulk_chunks(0)
            fb1 = None
            tp0 = None
            FB0_W = [3, 4, 5, 6, 7, 8]
            FB1_W = [9, 11, 12, 13, 14, 16]
            TP0_W = [11, 12, 13, 14, 15, 16]
            NG = NGROUPS
            for w in range(2 * NG + 3):
                for b, goff in ((0, 0), (1, NG)):
                    g = w - goff
                    if 0 <= g < NG:
                        stage_mm(b, g)
                        stage_ssum(b, g)
                if w == 2:
                    prelude(1)
                    fb1 = focal_bulk_chunks(1)
                if w == 10:
                    nc.sync.dma_start(
                        q_d[0].ap().rearrange("(p m) s -> p m s", p=P),
                        st[0]["q"][:])
                    st[0]["shipped"] = True
                for b, goff in ((0, 0), (1, NG)):
                    g = w - 1 - goff
                    if 0 <= g < NG:
                        stage_mid(b, g)
                for b, goff in ((0, 0), (1, NG)):
                    g = w - 2 - goff
                    if 0 <= g < NG:
                        stage_fin(b, g)
                    gq = w - 3 - goff
                    if 0 <= gq < NG:
                        stage_finq(b, gq)
                    gr = w - 1 - goff
                    if 0 <= gr < NG:
                        stage_recip(b, gr)
                if w == 10:
                    tp0 = tail_pieces(0)
                if w in FB0_W:
                    fb0[FB0_W.index(w)]()
                if w in FB1_W:
                    fb1[FB1_W.index(w)]()
                if w in TP0_W:
                    tp0[TP0_W.index(w)]()
            for p in tail_pieces(1):
                p()

    nc.compile()
    return nc


def _get_nc():
    if "nc" not in _cache:
        _cache["nc"] = _build()
    return _cache["nc"]


def kernel(preds: np.ndarray, targets: np.ndarray) -> np.ndarray:
    from concourse.bass_utils import run_bass_kernel_spmd

    nc = _get_nc()
    preds = np.ascontiguousarray(preds, dtype=np.float32)
    targets = np.ascontiguousarray(targets, dtype=np.float32)
    in_maps = []
    for c in range(N_CORES):
        s = c * IMGS_PER_CORE
        in_maps.append({"preds": preds[s:s + IMGS_PER_CORE],
                        "targets": targets[s:s + IMGS_PER_CORE]})
    res = run_bass_kernel_spmd(nc, in_maps, list(range(N_CORES)))
    per_image = np.concatenate([res.results[c]["out"] for c in range(N_CORES)])
    return np.float32(per_image.mean())


# revision 41
# speedup vs baseline: 1.0487x; 1.0487x over previous
"""Trainium2 Bass kernel for nn_DetectionLoss (B=16, N=25000, M=64).

Strategy (v4 — fp16 match pipeline, software-pipelined, short tail):
- Data-parallel: 8 cores x 2 images each; host shards batch and averages.
- Greedy match reformulated as per-GT argmax (exact). Ranking uses
  q = inter/(area_p+area_t), monotone in iou; thr is q > 1/6.
- Match DECISIONS tolerate fp16 (numpy sim: rel err 3.5e-4; gate 2e-2).
  The loss tail stays exact f32 via DRAM gathers of the raw preds.
- Bulk pairwise in fp16 for the DVE 2x_1p mode; broadcast target operands
  are materialized once per image as [P, M, UG] replicated tiles
  (log-doubling TensorCopy at 4x) so the mode is not lost to stride-0 APs.
- Engines: DVE minmax/inter/q/macc (f16 2x) + recip (f32); Pool ssum +
  dx/dy; Act relu + rsc f32->f16. Emission is wave-skewed (2-wave software
  pipeline); image 0's tail is interleaved into image 1's bulk waves.
- Tail avoids MaxIndex: first-occurrence argmax via is_equal + iota + min
  reduce (exact same tie-break as jnp.argmax on identical fp16 values).
  Focal correction f1-f0 is precomputed for all preds during bulk and
  gathered per matched GT. Cross-partition sums go through PE matmuls.
"""

import numpy as np

B, N, M = 16, 25000, 64
P = 128
SLOTS = 196
IMGS_PER_CORE = 2
N_CORES = 8
UG = 28
NGROUPS = SLOTS // UG  # 7

PAD_PART = 127
PAD_START = N - PAD_PART * SLOTS   # 108

_cache = {}


def _build(debug_dumps=False):
    import concourse.bass as bass
    import concourse.bacc as bacc
    import concourse.mybir as mybir
    from concourse import tile
    from concourse.bass import IndirectOffsetOnAxis
    from concourse.masks import make_identity

    f32 = mybir.dt.float32
    f16 = mybir.dt.float16
    u32 = mybir.dt.uint32
    i32 = mybir.dt.int32
    Alu = mybir.AluOpType
    Act = mybir.ActivationFunctionType
    X = mybir.AxisListType.X

    nc = bacc.Bacc("TRN2", target_bir_lowering=False, debug=False,
                   num_devices=N_CORES)

    preds_d = nc.dram_tensor("preds", [IMGS_PER_CORE, N, 5], f32, kind="ExternalInput")
    targets_d = nc.dram_tensor("targets", [IMGS_PER_CORE, M, 4], f32, kind="ExternalInput")
    out_d = nc.dram_tensor("out", [IMGS_PER_CORE], f32, kind="ExternalOutput")
    q_d = [nc.dram_tensor(f"q_scratch{b}", [P * M, SLOTS], f16)
           for b in range(IMGS_PER_CORE)]
    D_d = [nc.dram_tensor(f"d_scratch{b}", [P * SLOTS, 1], f32)
           for b in range(IMGS_PER_CORE)]

    EPS = np.float32(1e-7)
    C_4PI2 = np.float32(4.0 / (np.pi ** 2))
    BIG = float(2 ** 18)
    SP_SEED = [0.041064513, -0.156028432, 0.304672365, -0.496368282, 0.999887926]
    # A&S 4.4.49: atan(r)/r, poly in r^2, |err|<=1e-5 on [0,1]; plenty for
    # the aspect-ratio term (weight ~4/pi^2 * small delta)
    AT_POLY = [0.0208351, -0.0851330, 0.1801410, -0.3302995, 0.9998660]

    with tile.TileContext(nc) as tc:
        with (
            tc.tile_pool(name="qpool", bufs=2) as qpool,
            tc.tile_pool(name="ppool", bufs=2) as ppool,
            tc.tile_pool(name="der", bufs=2) as der,
            tc.tile_pool(name="rep", bufs=2) as rep,
            tc.tile_pool(name="grp", bufs=3) as grp,
            tc.tile_pool(name="qch", bufs=3) as qch,
            tc.tile_pool(name="ssp", bufs=2) as ssp,
            tc.tile_pool(name="inp", bufs=2) as inp,
            tc.tile_pool(name="mac", bufs=2) as mac,
            tc.tile_pool(name="sml", bufs=2) as sml,
            tc.tile_pool(name="cst", bufs=1) as cst,
            tc.tile_pool(name="psum", bufs=1,
                         space=bass.MemorySpace.PSUM) as psum,
        ):
            # constants
            iota_p64 = cst.tile([M, 1], i32, tag="iota_p64")
            nc.gpsimd.iota(iota_p64[:], pattern=[[1, 1]], base=0, channel_multiplier=1)
            iscr = cst.tile([M, SLOTS], i32, tag="iscr")
            nc.gpsimd.iota(iscr[:, :M], pattern=[[1, M]], base=0,
                           channel_multiplier=0)
            iota_p64f = cst.tile([M, 1], f32, tag="iota_p64f")
            nc.vector.tensor_copy(iota_p64f[:], iota_p64[:])
            iota_f64f = cst.tile([M, M], f32, tag="iota_f64f")
            nc.vector.tensor_copy(iota_f64f[:], iscr[:, :M])
            ltmask = cst.tile([M, M], f32, tag="ltmask")
            nc.vector.tensor_scalar(ltmask[:], iota_f64f[:], iota_p64f[:], None,
                                    op0=Alu.is_lt)
            iotaPf = cst.tile([M, P], f32, tag="iotaPf")
            nc.gpsimd.iota(iscr[:, :P], pattern=[[1, P]], base=0,
                           channel_multiplier=0)
            nc.vector.tensor_copy(iotaPf[:], iscr[:, :P])
            iotaSf = cst.tile([M, SLOTS], f32, tag="iotaSf")
            nc.gpsimd.iota(iscr[:], pattern=[[1, SLOTS]], base=0,
                           channel_multiplier=0)
            nc.vector.tensor_copy(iotaSf[:], iscr[:])
            ones_row = cst.tile([1, P], f32, tag="ones_row")
            nc.gpsimd.memset(ones_row[:], 1.0)
            ones_col64 = cst.tile([M, 1], f32, tag="ones_col64")
            nc.gpsimd.memset(ones_col64[:], 1.0)
            ones_col128 = cst.tile([P, 1], f32, tag="ones_col128")
            nc.gpsimd.memset(ones_col128[:], 1.0)
            ident = cst.tile([P, P], f32, tag="ident")
            make_identity(nc, ident[:])

            st = [dict() for _ in range(IMGS_PER_CORE)]

            def pesum(x_ap, n, tag):
                """sum over partitions of [n,1] x -> [1,1] f32 tile"""
                ps = psum.tile([1, 1], f32, tag="ps_sum", name="ps_sum")
                ones = ones_col64 if n == M else ones_col128
                nc.tensor.matmul(ps[:], x_ap, ones[:], start=True, stop=True)
                out = sml.tile([1, 1], f32, tag="sum_" + tag, name="sum_" + tag)
                nc.vector.tensor_copy(out[:], ps[:])
                return out

            # ---------------- loads for both images up front --------------
            for b in range(IMGS_PER_CORE):
                s = st[b]
                predsI = ppool.tile([P, SLOTS, 5], f32, tag="predsI", name="predsI")
                nc.gpsimd.memset(predsI[:, PAD_START:, 0:2], 50.0)
                nc.gpsimd.memset(predsI[:, PAD_START:, 2:4], 1e-4)
                nc.gpsimd.memset(predsI[:, PAD_START:, 4:5], -80.0)
                src = preds_d.ap()[b].rearrange("n c -> (n c)")
                nc.sync.dma_start(
                    predsI[:PAD_PART],
                    src[: PAD_PART * SLOTS * 5].rearrange("(p f) -> p f", p=PAD_PART)
                    .rearrange("p (s c) -> p s c", c=5))
                nc.sync.dma_start(
                    predsI[PAD_PART:, :PAD_START],
                    src[PAD_PART * SLOTS * 5:].rearrange("(p s c) -> p s c", p=1, c=5))
                s["predsI"] = predsI
                tg = sml.tile([M, 4], f32, tag="tg", name="tg")
                nc.sync.dma_start(tg[:], targets_d.ap()[b])
                trow = sml.tile([1, M, 4], f32, tag="trow", name="trow")
                nc.sync.dma_start(trow[:], targets_d.ap()[b].unsqueeze(0))
                s["tg"] = tg
                s["trow"] = trow

            # ---------------- prelude -------------------------------------
            def prelude(b):
                s = st[b]
                predsI = s["predsI"]
                wc = der.tile([P, SLOTS], f32, tag="wc", name="wc")
                hc = der.tile([P, SLOTS], f32, tag="hc", name="hc")
                x1p = der.tile([P, SLOTS], f16, tag="x1p", name="x1p")
                x2p = der.tile([P, SLOTS], f16, tag="x2p", name="x2p")
                y1p = der.tile([P, SLOTS], f16, tag="y1p", name="y1p")
                y2p = der.tile([P, SLOTS], f16, tag="y2p", name="y2p")
                apred = der.tile([P, SLOTS], f32, tag="apred", name="apred")
                nc.vector.tensor_scalar_max(wc[:], predsI[:, :, 2], 1e-4)
                nc.vector.tensor_scalar_max(hc[:], predsI[:, :, 3], 1e-4)
                nc.vector.tensor_tensor(apred[:], wc[:], hc[:], op=Alu.mult)
                nc.vector.tensor_scalar_mul(wc[:], wc[:], 0.5)
                nc.vector.tensor_tensor(x1p[:], predsI[:, :, 0], wc[:],
                                        op=Alu.subtract)
                nc.vector.tensor_tensor(x2p[:], predsI[:, :, 0], wc[:],
                                        op=Alu.add)
                nc.vector.tensor_scalar_mul(hc[:], hc[:], 0.5)
                nc.vector.tensor_tensor(y1p[:], predsI[:, :, 1], hc[:],
                                        op=Alu.subtract)
                nc.vector.tensor_tensor(y2p[:], predsI[:, :, 1], hc[:],
                                        op=Alu.add)
                s.update(x1p=x1p, x2p=x2p, y1p=y1p, y2p=y2p, apred=apred)

                trow = s["trow"]
                atrow = sml.tile([1, M, 2], f32, tag="atrow", name="atrow")
                nc.vector.tensor_sub(atrow[:, :, 0], trow[:, :, 2], trow[:, :, 0])
                nc.vector.tensor_sub(atrow[:, :, 1], trow[:, :, 3], trow[:, :, 1])
                nc.vector.tensor_tensor(atrow[:, :, 0], atrow[:, :, 0],
                                        atrow[:, :, 1], op=Alu.mult)
                coord16 = []
                for ci in range(4):
                    pt = psum.tile([P, M], f32, tag="bcast_ps", name="bcast_ps")
                    nc.tensor.matmul(pt[:], ones_row[:], trow[:, :, ci],
                                     start=True, stop=True)
                    c16 = rep.tile([P, M], f16, tag=f"tb16_{ci}", name=f"tb16_{ci}")
                    nc.scalar.copy(c16[:], pt[:])
                    coord16.append(c16)
                pt = psum.tile([P, M], f32, tag="bcast_ps", name="bcast_ps")
                nc.tensor.matmul(pt[:], ones_row[:], atrow[:, :, 0],
                                 start=True, stop=True)
                atB = rep.tile([P, M], f32, tag="atB", name="atB")
                nc.scalar.copy(atB[:], pt[:])
                s["atB"] = atB
                reps = []
                for ci in range(4):
                    cp = (nc.vector.tensor_copy if (b == 0 or ci >= 2)
                          else nc.scalar.copy)
                    r = rep.tile([P, M, UG], f16, tag=f"rep_{ci}", name=f"rep_{ci}")
                    cp(r[:, :, 0:1], coord16[ci][:].unsqueeze(2))
                    k = 1
                    while k < UG:
                        step = min(k, UG - k)
                        cp(r[:, :, k:k + step], r[:, :, 0:step])
                        k += step
                    reps.append(r)
                s["reps"] = reps
                s["q"] = qpool.tile([P, M, SLOTS], f16, tag="q", name="q")
                s["macc"] = mac.tile([P, M, UG], f16, tag="macc", name="macc")
                s["grp"] = {}
                s["qch"] = {}
                s["ssum"] = {}

            # ---------------- bulk wave stages ----------------------------
            def stage_mm(b, g):
                s = st[b]
                sl = slice(g * UG, (g + 1) * UG)
                x1tR, y1tR, x2tR, y2tR = s["reps"]

                def pv16(t):
                    return t[:, sl].unsqueeze(1).to_broadcast([P, M, UG])

                ltx = grp.tile([P, M, UG], f16, tag="ltx", name="ltx")
                rbx = grp.tile([P, M, UG], f16, tag="rbx", name="rbx")
                lty = grp.tile([P, M, UG], f16, tag="lty", name="lty")
                rby = grp.tile([P, M, UG], f16, tag="rby", name="rby")
                rsc16 = qch.tile([P, M, UG], f16, tag="rsc16", name="rsc16")
                intr = inp.tile([P, M, UG], f16, tag="intr", name="intr")
                s["grp"][g] = [ltx, rbx, lty, rby, None, rsc16, intr]
                nc.vector.tensor_tensor(ltx[:], pv16(s["x1p"]), x1tR[:], op=Alu.max)
                nc.vector.tensor_tensor(rbx[:], pv16(s["x2p"]), x2tR[:], op=Alu.min)
                nc.vector.tensor_tensor(lty[:], pv16(s["y1p"]), y1tR[:], op=Alu.max)
                nc.vector.tensor_tensor(rby[:], pv16(s["y2p"]), y2tR[:], op=Alu.min)

            def stage_ssum(b, g):
                s = st[b]
                sl = slice(g * UG, (g + 1) * UG)
                ssum = ssp.tile([P, M, UG], f32, tag="ssum", name="ssum")
                s["grp"][g][4] = ssum
                eng = nc.vector if (g < 3 if b == 0 else g < 1) else nc.gpsimd
                eng.tensor_tensor(
                    ssum[:],
                    s["apred"][:, sl].unsqueeze(1).to_broadcast([P, M, UG]),
                    s["atB"][:].unsqueeze(2).to_broadcast([P, M, UG]),
                    op=Alu.add)

            def stage_mid(b, g):
                s = st[b]
                ltx, rbx, lty, rby, ssum, rsc16, intr = s["grp"][g]
                nc.gpsimd.tensor_tensor(rbx[:], rbx[:], ltx[:], op=Alu.subtract)
                if b == 0 and g == NGROUPS - 1:
                    nc.vector.tensor_tensor(rby[:], rby[:], lty[:], op=Alu.subtract)
                else:
                    nc.gpsimd.tensor_tensor(rby[:], rby[:], lty[:], op=Alu.subtract)
                nc.scalar.activation(ltx[:], rbx[:], Act.Relu)

            def stage_recip(b, g):
                s = st[b]
                ltx, rbx, lty, rby, ssum, rsc16, intr = s["grp"][g]
                nc.vector.reciprocal_approx_fast(ssum[:], ssum[:])
                nc.scalar.copy(rsc16[:], ssum[:])

            def stage_fin(b, g):
                s = st[b]
                ltx, rbx, lty, rby, ssum, rsc16, intr = s["grp"][g]
                nc.vector.tensor_tensor(intr[:], ltx[:], rby[:], op=Alu.mult)
                s["qch"][g] = (rsc16, intr)
                del s["grp"][g]

            def stage_finq(b, g):
                s = st[b]
                rsc16, intr = s["qch"][g]
                sl = slice(g * UG, (g + 1) * UG)
                qs = s["q"][:, :, sl]
                nc.vector.tensor_tensor(qs, intr[:], rsc16[:], op=Alu.mult)
                if g == 0:
                    nc.vector.tensor_copy(s["macc"][:], qs)
                else:
                    nc.vector.tensor_tensor(s["macc"][:], s["macc"][:], qs,
                                            op=Alu.max)
                del s["qch"][g]

            # ---------------- focal bulk (chunked) ------------------------
            def focal_bulk_chunks(b):
                s = st[b]
                predsI = s["predsI"]
                tl = lambda t: der.tile([P, SLOTS], f32, tag="fb" + t, name="fb" + t)
                sg_, sp_, u_, w_, z_, e_ = (tl("sg"), tl("sp"), tl("u"),
                                            tl("w"), tl("z"), tl("e"))
                conf = predsI[:, :, 4]

                def c0():
                    nc.scalar.activation(e_[:], conf, Act.Exp, scale=-1.0)
                    nc.vector.tensor_scalar_add(e_[:], e_[:], 1.0)
                    nc.vector.reciprocal(sg_[:], e_[:])
                    nc.vector.tensor_scalar_mul(u_[:], conf, -1.0)
                    nc.vector.tensor_tensor(u_[:], u_[:], conf, op=Alu.max)
                    nc.scalar.activation(u_[:], u_[:], Act.Exp, scale=-1.0)
                    nc.vector.tensor_scalar_add(w_[:], u_[:], 1.0)

                def c1():
                    nc.vector.tensor_scalar(z_[:], u_[:], float(SP_SEED[0]),
                                            float(SP_SEED[1]), op0=Alu.mult,
                                            op1=Alu.add)
                    for coef in SP_SEED[2:]:
                        nc.vector.tensor_tensor(z_[:], z_[:], u_[:], op=Alu.mult)
                        nc.vector.tensor_scalar_add(z_[:], z_[:], float(coef))
                    nc.vector.tensor_tensor(z_[:], z_[:], u_[:], op=Alu.mult)

                def newton():
                    nc.scalar.activation(e_[:], z_[:], Act.Exp, scale=-1.0)
                    nc.vector.tensor_tensor(e_[:], w_[:], e_[:], op=Alu.mult)
                    nc.vector.tensor_tensor(z_[:], z_[:], e_[:], op=Alu.add)
                    nc.vector.tensor_scalar_add(z_[:], z_[:], -1.0)

                def c3():
                    nc.scalar.activation(sp_[:], conf, Act.Relu)
                    nc.vector.tensor_add(sp_[:], sp_[:], z_[:])
                    f0 = z_  # f0 = sg^2 * sp
                    nc.vector.tensor_tensor(f0[:], sg_[:], sg_[:], op=Alu.mult)
                    nc.vector.tensor_tensor(f0[:], f0[:], sp_[:], op=Alu.mult)
                    frow = sml.tile([P, 1], f32, tag="frow", name="frow")
                    nc.vector.tensor_reduce(frow[:], f0[:], axis=X, op=Alu.add)
                    s["fsum"] = pesum(frow[:], P, f"fs{b}")

                def c4():
                    # D = 0.25*(1-sg)^2*(sp-conf) - 0.75*f0  (focal1 - focal0)
                    t1_, t2_ = u_, e_
                    nc.vector.tensor_scalar(t1_[:], sg_[:], -1.0, 1.0,
                                            op0=Alu.mult, op1=Alu.add)
                    nc.vector.tensor_tensor(t1_[:], t1_[:], t1_[:], op=Alu.mult)
                    nc.vector.tensor_tensor(t2_[:], sp_[:], conf, op=Alu.subtract)
                    nc.vector.tensor_tensor(t1_[:], t1_[:], t2_[:], op=Alu.mult)
                    nc.vector.tensor_scalar_mul(t1_[:], t1_[:], 0.25)
                    nc.vector.tensor_scalar_mul(z_[:], z_[:], 0.75)  # z_ holds f0
                    nc.vector.tensor_tensor(t1_[:], t1_[:], z_[:], op=Alu.subtract)
                    nc.sync.dma_start(
                        D_d[b].ap().rearrange("(p s) o -> p s o", p=P),
                        t1_[:].unsqueeze(2))

                return [c0, c1, newton, newton, c3, c4]

            # ---------------- tail pieces ---------------------------------
            def tail_pieces(b):
                s = st[b]
                tg = s["tg"]
                h = {}
                t1 = lambda tag: sml.tile([M, 1], f32, tag=tag, name=tag)
                t2_ = lambda tag: sml.tile([M, 2], f32, tag=tag, name=tag)

                def p0():
                    if "shipped" not in st[b]:
                        nc.sync.dma_start(
                            q_d[b].ap().rearrange("(p m) s -> p m s", p=P),
                            st[b]["q"][:])
                        st[b]["shipped"] = True
                    m1 = sml.tile([P, M], f32, tag="m1", name="m1")
                    nc.vector.tensor_reduce(m1[:], s["macc"][:], axis=X, op=Alu.max)
                    m1tp = psum.tile([M, P], f32, tag="m1tp", name="m1tp")
                    nc.tensor.transpose(m1tp[:], m1[:], ident[:])
                    m1t = sml.tile([M, P], f32, tag="m1t", name="m1t")
                    nc.vector.tensor_copy(m1t[:], m1tp[:])
                    gmax = t1("gmax")
                    nc.vector.tensor_reduce(gmax[:], m1t[:], axis=X, op=Alu.max)
                    # p* = first partition hitting gmax: is_eq*(-BIG)+iota, min
                    nc.vector.tensor_scalar(m1t[:], m1t[:], gmax[:], -BIG,
                                            op0=Alu.is_equal, op1=Alu.mult)
                    nc.vector.tensor_tensor(m1t[:], m1t[:], iotaPf[:], op=Alu.add)
                    pstar = t1("pstar")
                    nc.vector.tensor_reduce(pstar[:], m1t[:], axis=X, op=Alu.min)
                    nc.vector.tensor_scalar_add(pstar[:], pstar[:], BIG)
                    pu = sml.tile([M, 1], u32, tag="pu", name="pu")
                    nc.vector.tensor_copy(pu[:], pstar[:])
                    rowoff = sml.tile([M, 1], u32, tag="rowoff", name="rowoff")
                    nc.vector.tensor_scalar_mul(rowoff[:], pu[:], M)
                    nc.vector.tensor_tensor(rowoff[:], rowoff[:],
                                            iota_p64[:].bitcast(u32), op=Alu.add)
                    qrow16 = sml.tile([M, SLOTS], f16, tag="qrow16", name="qrow16")
                    nc.gpsimd.indirect_dma_start(
                        out=qrow16[:], out_offset=None,
                        in_=q_d[b].ap(),
                        in_offset=IndirectOffsetOnAxis(ap=rowoff[:], axis=0))
                    h.update(gmax=gmax, pstar=pstar, qrow16=qrow16)

                def p1():
                    qrow = sml.tile([M, SLOTS], f32, tag="qrow", name="qrow")
                    nc.vector.tensor_copy(qrow[:], h["qrow16"][:])
                    # c* = first slot hitting gmax within the gathered row
                    nc.vector.tensor_scalar(qrow[:], qrow[:], h["gmax"][:], -BIG,
                                            op0=Alu.is_equal, op1=Alu.mult)
                    nc.vector.tensor_tensor(qrow[:], qrow[:], iotaSf[:], op=Alu.add)
                    cstar = t1("cstar")
                    nc.vector.tensor_reduce(cstar[:], qrow[:], axis=X, op=Alu.min)
                    nc.vector.tensor_scalar_add(cstar[:], cstar[:], BIG)
                    nstar_f = t1("nstar_f")
                    nc.vector.tensor_scalar(nstar_f[:], h["pstar"][:], float(SLOTS),
                                            cstar[:], op0=Alu.mult, op1=Alu.add)
                    nstar = sml.tile([M, 1], u32, tag="nstar", name="nstar")
                    nc.vector.tensor_copy(nstar[:], nstar_f[:])
                    thr = t1("thr")
                    nc.vector.tensor_scalar(thr[:], h["gmax"][:], float(1.0 / 6.0),
                                            None, op0=Alu.is_gt)
                    # start g5 + D gathers (independent of dedup)
                    g5 = sml.tile([M, 5], f32, tag="g5", name="g5")
                    nrow = sml.tile([M, 1], u32, tag="nrow", name="nrow")
                    nc.vector.tensor_scalar_add(nrow[:], nstar[:], b * N)
                    nc.gpsimd.indirect_dma_start(
                        out=g5[:], out_offset=None,
                        in_=preds_d.ap().rearrange("b n c -> (b n) c"),
                        in_offset=IndirectOffsetOnAxis(ap=nrow[:], axis=0))
                    Dg = sml.tile([M, 1], f32, tag="Dg", name="Dg")
                    nc.gpsimd.indirect_dma_start(
                        out=Dg[:], out_offset=None,
                        in_=D_d[b].ap(),
                        in_offset=IndirectOffsetOnAxis(ap=nstar[:], axis=0))
                    h.update(nstar_f=nstar_f, thr=thr, g5=g5, Dg=Dg)

                def p2():
                    nstar_f, thr = h["nstar_f"], h["thr"]
                    pair = sml.tile([M, 2], f32, tag="pair", name="pair")
                    nc.vector.tensor_copy(pair[:, 0:1], nstar_f[:])
                    nc.vector.tensor_copy(pair[:, 1:2], thr[:])
                    pairT_ps = psum.tile([1, 2, M], f32, tag="pairT_ps",
                                         name="pairT_ps")
                    nc.tensor.transpose(pairT_ps[:, 0], pair[:, 0:1], ident[:M, :M])
                    nc.tensor.transpose(pairT_ps[:, 1], pair[:, 1:2], ident[:M, :M])
                    pairT = sml.tile([1, 2, M], f32, tag="pairT", name="pairT")
                    nc.vector.tensor_copy(pairT[:], pairT_ps[:])
                    rowB = sml.tile([M, M, 2], f32, tag="rowB", name="rowB")
                    ptb = psum.tile([M, M, 2], f32, tag="ptb", name="ptb")
                    nc.tensor.matmul(ptb[:, :, 0], ones_row[:, :M], pairT[:, 0],
                                     start=True, stop=True)
                    nc.tensor.matmul(ptb[:, :, 1], ones_row[:, :M], pairT[:, 1],
                                     start=True, stop=True)
                    nc.scalar.copy(rowB[:], ptb[:])
                    eq = sml.tile([M, M], f32, tag="eq", name="eq")
                    nc.vector.tensor_scalar(eq[:], rowB[:, :, 0], nstar_f[:], None,
                                            op0=Alu.is_equal)
                    nc.vector.tensor_tensor(eq[:], eq[:], rowB[:, :, 1], op=Alu.mult)
                    nc.vector.tensor_tensor(eq[:], eq[:], ltmask[:], op=Alu.mult)
                    blocked = t1("blocked")
                    nc.vector.tensor_reduce(blocked[:], eq[:], axis=X, op=Alu.max)
                    ok = t1("ok")
                    nc.vector.tensor_scalar(ok[:], blocked[:], -1.0, 1.0,
                                            op0=Alu.mult, op1=Alu.add)
                    nc.vector.tensor_tensor(ok[:], ok[:], thr[:], op=Alu.mult)
                    h["ok"] = ok

                def p3():
                    # lane-packed ciou: lane 0 = x, lane 1 = y
                    g5 = h["g5"]
                    tgA, tgB = tg[:, 0:2], tg[:, 2:4]     # (tx1,ty1), (tx2,ty2)
                    wh2 = t2_("wh2")
                    nc.vector.tensor_scalar_max(wh2[:], g5[:, 2:4], 1e-4)
                    nc.vector.tensor_scalar_mul(wh2[:], wh2[:], 0.5)
                    c1_ = t2_("c1_")
                    c2_ = t2_("c2_")
                    nc.vector.tensor_tensor(c1_[:], g5[:, 0:2], wh2[:],
                                            op=Alu.subtract)
                    nc.vector.tensor_tensor(c2_[:], g5[:, 0:2], wh2[:], op=Alu.add)
                    lt2 = t2_("lt2")
                    rb2 = t2_("rb2")
                    nc.vector.tensor_tensor(lt2[:], c1_[:], tgA, op=Alu.max)
                    nc.vector.tensor_tensor(rb2[:], c2_[:], tgB, op=Alu.min)
                    nc.vector.tensor_tensor(rb2[:], rb2[:], lt2[:], op=Alu.subtract)
                    nc.vector.tensor_scalar_max(rb2[:], rb2[:], 0.0)
                    ginter = t1("ginter")
                    nc.vector.tensor_tensor(ginter[:], rb2[:, 0:1], rb2[:, 1:2],
                                            op=Alu.mult)
                    whp = t2_("whp")
                    wht = t2_("wht")
                    nc.vector.tensor_tensor(whp[:], c2_[:], c1_[:], op=Alu.subtract)
                    nc.vector.tensor_tensor(wht[:], tgB, tgA, op=Alu.subtract)
                    gu = t1("gu")
                    ga = t1("ga")
                    nc.vector.tensor_tensor(gu[:], whp[:, 0:1], whp[:, 1:2],
                                            op=Alu.mult)
                    nc.vector.tensor_tensor(ga[:], wht[:, 0:1], wht[:, 1:2],
                                            op=Alu.mult)
                    nc.vector.tensor_add(gu[:], gu[:], ga[:])
                    nc.vector.tensor_sub(gu[:], gu[:], ginter[:])
                    giou = t1("giou")
                    nc.vector.tensor_scalar_add(gu[:], gu[:], float(EPS))
                    nc.vector.reciprocal(gu[:], gu[:])
                    nc.vector.tensor_tensor(giou[:], ginter[:], gu[:], op=Alu.mult)
                    h.update(c1_=c1_, c2_=c2_, giou=giou, whp=whp, wht=wht)

                def p4():
                    c1_, c2_ = h["c1_"], h["c2_"]
                    giou, whp, wht = h["giou"], h["whp"], h["wht"]
                    tgA, tgB = tg[:, 0:2], tg[:, 2:4]
                    e1 = t2_("e1")
                    e2 = t2_("e2")
                    nc.vector.tensor_tensor(e1[:], c1_[:], tgA, op=Alu.min)
                    nc.vector.tensor_tensor(e2[:], c2_[:], tgB, op=Alu.max)
                    nc.vector.tensor_tensor(e2[:], e2[:], e1[:], op=Alu.subtract)
                    nc.vector.tensor_tensor(e2[:], e2[:], e2[:], op=Alu.mult)
                    diag = t1("diag")
                    nc.vector.tensor_add(diag[:], e2[:, 0:1], e2[:, 1:2])
                    nc.vector.tensor_scalar_add(diag[:], diag[:], float(EPS))
                    ce2 = t2_("ce2")
                    nc.vector.tensor_tensor(ce2[:], c1_[:], c2_[:], op=Alu.add)
                    nc.vector.tensor_tensor(ce2[:], ce2[:], tgA, op=Alu.subtract)
                    nc.vector.tensor_tensor(ce2[:], ce2[:], tgB, op=Alu.subtract)
                    nc.vector.tensor_tensor(ce2[:], ce2[:], ce2[:], op=Alu.mult)
                    cent = t1("cent")
                    nc.vector.tensor_add(cent[:], ce2[:, 0:1], ce2[:, 1:2])
                    nc.vector.tensor_scalar_mul(cent[:], cent[:], 0.25)
                    diou = t1("diou")
                    nc.vector.reciprocal(diag[:], diag[:])
                    nc.vector.tensor_tensor(diou[:], cent[:], diag[:], op=Alu.mult)
                    nc.vector.tensor_sub(diou[:], diou[:], giou[:])
                    nc.vector.tensor_scalar_add(diou[:], diou[:], 1.0)
                    # v term: atan ratios packed [t, p]
                    rat = sml.tile([M, 2], f32, tag="rat", name="rat")
                    big2 = sml.tile([M, 2], i32, tag="big2", name="big2")
                    inv2 = sml.tile([M, 2], f32, tag="inv2", name="inv2")
                    s2 = sml.tile([M, 2], f32, tag="s2", name="s2")
                    ac2 = sml.tile([M, 2], f32, tag="ac2", name="ac2")
                    nc.vector.reciprocal(rat[:, 0:1], wht[:, 1:2])
                    nc.vector.tensor_tensor(rat[:, 0:1], wht[:, 0:1], rat[:, 0:1],
                                            op=Alu.mult)
                    nc.vector.reciprocal(rat[:, 1:2], whp[:, 1:2])
                    nc.vector.tensor_tensor(rat[:, 1:2], whp[:, 0:1], rat[:, 1:2],
                                            op=Alu.mult)
                    nc.vector.tensor_scalar(big2[:], rat[:], 1.0, None, op0=Alu.is_gt)
                    nc.vector.reciprocal(inv2[:], rat[:])
                    nc.vector.copy_predicated(rat[:], big2[:], inv2[:])
                    nc.vector.tensor_tensor(s2[:], rat[:], rat[:], op=Alu.mult)
                    nc.vector.tensor_scalar(ac2[:], s2[:], float(AT_POLY[0]),
                                            float(AT_POLY[1]), op0=Alu.mult,
                                            op1=Alu.add)
                    for coef in AT_POLY[2:]:
                        nc.vector.tensor_tensor(ac2[:], ac2[:], s2[:], op=Alu.mult)
                        nc.vector.tensor_scalar_add(ac2[:], ac2[:], float(coef))
                    nc.vector.tensor_tensor(ac2[:], ac2[:], rat[:], op=Alu.mult)
                    nc.vector.tensor_scalar(inv2[:], ac2[:], -1.0, float(np.pi / 2),
                                            op0=Alu.mult, op1=Alu.add)
                    nc.vector.copy_predicated(ac2[:], big2[:], inv2[:])
                    vv = t1("vv")
                    nc.vector.tensor_sub(vv[:], ac2[:, 0:1], ac2[:, 1:2])
                    nc.vector.tensor_tensor(vv[:], vv[:], vv[:], op=Alu.mult)
                    nc.vector.tensor_scalar_mul(vv[:], vv[:], float(C_4PI2))
                    av = t1("av")
                    nc.vector.tensor_scalar(av[:], giou[:], -1.0, float(1.0 + EPS),
                                            op0=Alu.mult, op1=Alu.add)
                    nc.vector.tensor_add(av[:], av[:], vv[:])
                    nc.vector.reciprocal(av[:], av[:])
                    nc.vector.tensor_tensor(av[:], av[:], vv[:], op=Alu.mult)
                    ciou = t1("ciou")
                    nc.vector.tensor_tensor(ciou[:], av[:], vv[:], op=Alu.mult)
                    nc.vector.tensor_add(ciou[:], ciou[:], diou[:])
                    ok = h["ok"]
                    nc.vector.tensor_tensor(ciou[:], ciou[:], ok[:], op=Alu.mult)
                    bsum = pesum(ciou[:], M, f"bs{b}")
                    nmatch = pesum(ok[:], M, f"nm{b}")
                    nc.vector.tensor_scalar_max(nmatch[:], nmatch[:], 1.0)
                    nc.vector.reciprocal(nmatch[:], nmatch[:])
                    box_loss = sml.tile([1, 1], f32, tag="box_loss", name="box_loss")
                    nc.vector.tensor_tensor(box_loss[:], bsum[:], nmatch[:],
                                            op=Alu.mult)
                    h["box_loss"] = box_loss

                def p5():
                    s_ = st[b]
                    mf1 = t1("mf1")
                    nc.vector.tensor_tensor(mf1[:], h["Dg"][:], h["ok"][:],
                                            op=Alu.mult)
                    dsum = pesum(mf1[:], M, f"ds{b}")
                    acc = sml.tile([1, 1], f32, tag="acc", name="acc")
                    nc.vector.tensor_scalar_mul(acc[:], s_["fsum"][:], 0.75)
                    nc.vector.tensor_add(acc[:], acc[:], dsum[:])
                    nc.vector.tensor_scalar_mul(acc[:], acc[:], float(1.0 / N))
                    nc.vector.tensor_add(acc[:], acc[:], h["box_loss"][:])
                    nc.sync.dma_start(out_d.ap()[b:b + 1],
                                      acc[:].rearrange("o m -> (o m)"))

                return [p0, p1, p2, p3, p4, p5]

            # ---------------- emission schedule ---------------------------
            # unified wave schedule: image 0 groups at waves 0..6, image 1
            # at waves 7..13 (Pool stream seamless across the transition)
            prelude(0)
            fb0 = focal_bulk_chunks(0)
            fb1 = None
            tp0 = None
            FB0_W = [3, 4, 5, 6, 7, 8]
            FB1_W = [9, 11, 12, 13, 14, 16]
            TP0_W = [11, 12, 13, 14, 15, 16]
            NG = NGROUPS
            for w in range(2 * NG + 3):
                for b, goff in ((0, 0), (1, NG)):
                    g = w - goff
                    if 0 <= g < NG:
                        stage_mm(b, g)
                        stage_ssum(b, g)
                if w == 2:
                    prelude(1)
                    fb1 = focal_bulk_chunks(1)
                if w == 10:
                    nc.sync.dma_start(
                        q_d[0].ap().rearrange("(p m) s -> p m s", p=P),
                        st[0]["q"][:])
                    st[0]["shipped"] = True
                for b, goff in ((0, 0), (1, NG)):
                    g = w - 1 - goff
                    if 0 <= g < NG:
                        stage_mid(b, g)
                for b, goff in ((0, 0), (1, NG)):
                    g = w - 2 - goff
                    if 0 <= g < NG:
                        stage_fin(b, g)
                    gq = w - 3 - goff
                    if 0 <= gq < NG:
                        stage_finq(b, gq)
                    gr = w - 1 - goff
                    if 0 <= gr < NG:
                        stage_recip(b, gr)
                if w == 10:
                    tp0 = tail_pieces(0)
                if w in FB0_W:
                    fb0[FB0_W.index(w)]()
                if w in FB1_W:
                    fb1[FB1_W.index(w)]()
                if w in TP0_W:
                    tp0[TP0_W.index(w)]()
            for p in tail_pieces(1):
                p()

    nc.compile()
    return nc


def _get_nc():
    if "nc" not in _cache:
        _cache["nc"] = _build()
    return _cache["nc"]


def kernel(preds: np.ndarray, targets: np.ndarray) -> np.ndarray:
    from concourse.bass_utils import run_bass_kernel_spmd

    nc = _get_nc()
    preds = np.ascontiguousarray(preds, dtype=np.float32)
    targets = np.ascontiguousarray(targets, dtype=np.float32)
    in_maps = []
    for c in range(N_CORES):
        s = c * IMGS_PER_CORE
        in_maps.append({"preds": preds[s:s + IMGS_PER_CORE],
                        "targets": targets[s:s + IMGS_PER_CORE]})
    res = run_bass_kernel_spmd(nc, in_maps, list(range(N_CORES)))
    per_image = np.concatenate([res.results[c]["out"] for c in range(N_CORES)])
    return np.float32(per_image.mean())
